# revision 9
# baseline (speedup 1.0000x reference)
"""Trainium2 Bass kernel for nn_AttentionBlock (B=4, L=S=1024, DIM=1024, NH=16).

Sharding: 8 cores = (batch b = core//2) x (head-half hh = core%2, 8 heads each).
Each core computes its batch's QKV projections restricted to its 512 feature
columns, attention for its 8 heads, and a partial output projection
(Wp row-slice); the host sums the two partials per batch.

Device layout is fully transposed ("T" = features/S on partitions) so no
on-device transposes are needed:
  qhT/khT (feat, L|S) from  Wslice.T @ xT ;  scoresT (S, L) = khT.T-slice @ qhT
  pos_bias enters PSUM via identity-matmul accumulation; mask is applied
  multiplicatively post-exp (host pre-merges causal+row-fix into the mask);
  softmax denominators ride a ones-column appended to V; normalization happens
  on the small (64, L) PV output via a rank-1 PE broadcast of 1/denom.
Compute dtype bf16 (f32 PSUM accumulation), f32 partial outputs.
"""
import contextlib
import ctypes
import sys
import types

import numpy as np
import ml_dtypes

bf16 = ml_dtypes.bfloat16

B, L, S, DIM, NH, DH = 4, 1024, 1024, 1024, 16, 64
NHC = 8           # heads per core
DIMC = 512        # feature columns per core
SCALE = 1.0 / np.sqrt(DH).astype(np.float32)

TRACE = False          # test.py flips this for profiling runs
TRACE_DIR = None
LAST_EXEC_NS = None


# ---------------------------------------------------------------- env setup
def _install_ntff_hook():
    if "antenv.axon_hooks" in sys.modules:
        return
    try:
        lib = ctypes.CDLL("/opt/axon/libaxon_pjrt.so")
        lib.axon_start_nrt_profile.argtypes = [
            ctypes.POINTER(ctypes.c_int64),
            ctypes.c_size_t,
        ]
        lib.axon_start_nrt_profile.restype = ctypes.c_int64
        lib.axon_stop_nrt_profile.argtypes = [ctypes.c_char_p]
        lib.axon_stop_nrt_profile.restype = ctypes.c_int64
    except OSError:
        return

    @contextlib.contextmanager
    def _hook(output_dir, device_ids):
        import jax

        jax.devices()
        if device_ids:
            ids = (ctypes.c_int64 * len(device_ids))(*device_ids)
            rc = lib.axon_start_nrt_profile(ids, len(device_ids))
        else:
            rc = lib.axon_start_nrt_profile(None, 0)
        if rc != 0:
            raise RuntimeError(f"axon_start_nrt_profile rc={rc}")
        try:
            yield
        finally:
            n = lib.axon_stop_nrt_profile(str(output_dir).encode())
            print(f"profile: {n} file(s) written to {output_dir}")

    mod = types.ModuleType("antenv.axon_hooks")
    mod.get_axon_ntff_profile_hook = lambda: _hook
    mod.set_axon_ntff_profile_hook = lambda h: None
    sys.modules["antenv.axon_hooks"] = mod


def _patch_tile_drain():
    from concourse import mybir
    from concourse.tile import TileContext, ScopedClock

    if getattr(TileContext, "_drain_split_patched", False):
        return

    def _drain_and_barrier(self, tick_clock, wait_clock):
        drain_inst = self.nc.sync.drain()
        wait_clock.add_sem_waits(
            drain_inst.ins, ScopedClock({None: tick_clock.global_clock})
        )
        waits = list(drain_inst.ins.sync_info.on_wait)
        if len(waits) > 1:
            drain_inst.ins.sync_info.on_wait = waits[:1]
            for w in waits[1:]:
                nop = self.nc.sync.nop()
                nop.ins.sync_info = mybir.SyncInfo(on_wait=[w], on_update=[])
        self.nc.all_engine_barrier()
        assert self.sems is not None
        popped = self.nc._tile_sem_poison_stack.pop()
        assert popped is self._sem_poison
        self.nc.clear_and_free_semaphores(list(self.sems.allocated().values()))
        self.nc.all_engine_barrier()

    TileContext._drain_and_barrier = _drain_and_barrier
    TileContext._drain_split_patched = True


def _split_multiwait_instructions(nc):
    """This container's walrus rejects >1 sync wait per instruction; hoist
    extras onto same-engine NOPs placed right before the instruction."""
    from concourse import mybir

    n_split = 0
    for fn in nc.m.functions:
        for bb in fn.blocks:
            out = []
            for inst in bb.instructions:
                si = inst.sync_info
                waits = list(si.on_wait) if si is not None else []
                if len(waits) > 1:
                    for w in waits[:-1]:
                        n_split += 1
                        out.append(
                            mybir.InstNoOp(
                                name=f"waitsplit-{n_split}-{inst.name}",
                                engine=inst.engine,
                                bass_nofuse=True,
                                sync_info=mybir.SyncInfo(on_wait=[w], on_update=[]),
                            )
                        )
                    si.on_wait = waits[-1:]
                out.append(inst)
            if n_split:
                bb.instructions = out


# ---------------------------------------------------------------- builder
_NC_CACHE = {}


def build_nc(use_bq=False, use_bk=False, use_bv=False, use_bp=False):
    key = (use_bq, use_bk, use_bv, use_bp)
    if key in _NC_CACHE:
        return _NC_CACHE[key]
    _install_ntff_hook()
    _patch_tile_drain()
    import concourse.bass as bass
    import concourse.tile as tile
    from concourse import mybir

    dt = mybir.dt
    AF = mybir.ActivationFunctionType

    nc = bass.Bass("TRN2", target_bir_lowering=False, debug=False, num_devices=8)

    qT_d = nc.declare_dram_parameter("qT", (DIM, L), dt.bfloat16, isOutput=False)
    kT_d = nc.declare_dram_parameter("kT", (DIM, S), dt.bfloat16, isOutput=False)
    vT_d = nc.declare_dram_parameter("vT", (DIM, S), dt.bfloat16, isOutput=False)
    wq_d = nc.declare_dram_parameter("wq", (DIM, DIMC), dt.bfloat16, isOutput=False)
    wk_d = nc.declare_dram_parameter("wk", (DIM, DIMC), dt.bfloat16, isOutput=False)
    wv_d = nc.declare_dram_parameter("wv", (DIM, DIMC), dt.bfloat16, isOutput=False)
    wp_d = nc.declare_dram_parameter("wp", (DIMC, DIM), dt.bfloat16, isOutput=False)
    pb_d = nc.declare_dram_parameter("pbT", (NHC * S, L), dt.bfloat16, isOutput=False)
    mf_d = nc.declare_dram_parameter("mfT", (S, L), dt.bfloat16, isOutput=False)
    bq_d = nc.declare_dram_parameter("bq", (1, DIMC), dt.float32, isOutput=False)
    bk_d = nc.declare_dram_parameter("bk", (1, DIMC), dt.float32, isOutput=False)
    bv_d = nc.declare_dram_parameter("bv", (1, DIMC), dt.float32, isOutput=False)
    bp_d = nc.declare_dram_parameter("bp", (128, 8), dt.float32, isOutput=False)
    id_d = nc.declare_dram_parameter("ident", (128, 128), dt.bfloat16, isOutput=False)
    out_d = nc.declare_dram_parameter("out", (DIM, L), dt.float32, isOutput=True)

    with tile.TileContext(nc) as tc:
        with (
            tc.tile_pool(name="consts", bufs=1) as consts,
            tc.tile_pool(name="w", bufs=1) as wpool,
            tc.tile_pool(name="mf", bufs=1) as mfpool,
            tc.tile_pool(name="heads", bufs=1) as heads,
            tc.tile_pool(name="stage", bufs=1) as stage,
            tc.tile_pool(name="ostage", bufs=4) as ostage,
        ):
            ones_t = consts.tile([128, 64], dt.bfloat16)
            nc.gpsimd.memset(ones_t[:], 1.0)
            if use_bq:
                bq_t = consts.tile([1, DIMC], dt.float32)
                nc.sync.dma_start(bq_t[:], bq_d[:])
            if use_bk:
                bk_t = consts.tile([1, DIMC], dt.float32)
                nc.sync.dma_start(bk_t[:], bk_d[:])
            if use_bv:
                bv_t = consts.tile([1, DIMC], dt.float32)
                nc.sync.dma_start(bv_t[:], bv_d[:])
                ones_f = consts.tile([1, 128], dt.float32)
                nc.gpsimd.memset(ones_f[:], 1.0)
            if use_bq or use_bk:
                ones_r = consts.tile([1, 512], dt.float32)
                nc.gpsimd.memset(ones_r[:], 1.0)

            # big consolidated weight tiles: w*[:, dt*512 + p*128 + ...] = W[dt*128+p-row, col]
            wq_t = wpool.tile([128, 8 * DIMC], dt.bfloat16, name="wqb", tag="wqb")
            wk_t = wpool.tile([128, 8 * DIMC], dt.bfloat16, name="wkb", tag="wkb")
            wv_t = wpool.tile([128, 8 * DIMC], dt.bfloat16, name="wvb", tag="wvb")
            wp_t = wpool.tile([128, 4 * DIM], dt.bfloat16, name="wpb", tag="wpb")
            mf_t = mfpool.tile([128, 8 * L], dt.bfloat16, name="mfb", tag="mfb")

            qh_t = [heads.tile([128, L], dt.bfloat16, name=f"qh{i}", tag=f"qh{i}") for i in range(4)]
            kh_t = [heads.tile([128, S], dt.bfloat16, name=f"kh{i}", tag=f"kh{i}") for i in range(4)]
            vh_t = [heads.tile([128, NHC * 65], dt.bfloat16, name=f"vh{i}", tag=f"vh{i}") for i in range(8)]
            oT_t = [heads.tile([128, L], dt.bfloat16, name=f"oT{i}", tag=f"oT{i}") for i in range(4)]

            def load_big(tile_ap, dram, rows, cols, chunks=1):
                # tile[:, a*cols + c] = dram[a*128 + p, c]
                n_a = rows // 128
                a_per = n_a // chunks
                for ch in range(chunks):
                    nc.sync.dma_start(
                        tile_ap[:, ch * a_per * cols : (ch + 1) * a_per * cols]
                        .rearrange("p (a c) -> p a c", c=cols),
                        dram[ch * a_per * 128 : (ch + 1) * a_per * 128, :]
                        .rearrange("(a p) c -> p a c", p=128),
                    )

            # ================= phase B: projections =================
            with (
                tc.tile_pool(name="xT", bufs=1) as xTp,
                tc.tile_pool(name="projps", bufs=4, space="PSUM") as pps,
            ):
                xb = {}
                for nm in ("q", "k", "v"):
                    xb[nm] = xTp.tile([128, 8 * 1024], dt.bfloat16, name=f"{nm}Tb", tag=f"{nm}Tb")

                load_big(wq_t, wq_d, DIM, DIMC)
                load_big(xb["q"], qT_d, DIM, L, chunks=2)
                load_big(wk_t, wk_d, DIM, DIMC)
                load_big(xb["k"], kT_d, DIM, S, chunks=2)
                id_t = consts.tile([128, 128], dt.bfloat16)
                nc.sync.dma_start(id_t[:], id_d[:])
                load_big(mf_t, mf_d, S, L)

                # Q+K projections interleaved per head-pair so pair 0 finishes
                # early and attention can start while pairs 1-3 project
                for p in range(4):
                    psq = pps.tile([128, 1024], dt.float32, name=f"psq{p}", tag="ps")
                    psk = pps.tile([128, 1024], dt.float32, name=f"psk{p}", tag="ps")
                    for dtile in range(8):
                        for ps, w_t, x_t in ((psq, wq_t, xb["q"]), (psk, wk_t, xb["k"])):
                            for lc in range(2):
                                nc.tensor.matmul(
                                    ps[:, lc * 512 : (lc + 1) * 512],
                                    w_t[:, dtile * 512 + p * 128 : dtile * 512 + (p + 1) * 128],
                                    x_t[:, dtile * 1024 + lc * 512 : dtile * 1024 + (lc + 1) * 512],
                                    start=(dtile == 0),
                                    stop=(dtile == 7)
                                    and not (use_bq if w_t is wq_t else use_bk),
                                )
                    if use_bq:
                        for lc in range(2):
                            nc.tensor.matmul(
                                psq[:, lc * 512 : (lc + 1) * 512],
                                bq_t[0:1, p * 128 : (p + 1) * 128],
                                ones_r[0:1, 0:512],
                                start=False, stop=True,
                            )
                    if use_bk:
                        for lc in range(2):
                            nc.tensor.matmul(
                                psk[:, lc * 512 : (lc + 1) * 512],
                                bk_t[0:1, p * 128 : (p + 1) * 128],
                                ones_r[0:1, 0:512],
                                start=False, stop=True,
                            )
                    nc.vector.tensor_copy(qh_t[p][:], psq[:])
                    nc.vector.tensor_copy(kh_t[p][:], psk[:])

                # V: normal layout (S on partitions), ones column interleaved
                load_big(xb["v"], vT_d, DIM, S, chunks=2)
                load_big(wv_t, wv_d, DIM, DIMC)
                psv = [pps.tile([128, 1024], dt.float32, name=f"psv{i}", tag="ps") for i in range(4)]
                for dtile in range(8):
                    for i in range(4):
                        for half in range(2):
                            st = 2 * i + half
                            nc.tensor.matmul(
                                psv[i][:, half * 512 : (half + 1) * 512],
                                xb["v"][:, dtile * 1024 + st * 128 : dtile * 1024 + (st + 1) * 128],
                                wv_t[:, dtile * 512 : (dtile + 1) * 512],
                                start=(dtile == 0),
                                stop=(dtile == 7) and not use_bv,
                            )
                for i in range(4):
                    for half in range(2):
                        st = 2 * i + half
                        if use_bv:
                            nc.tensor.matmul(
                                psv[i][:, half * 512 : (half + 1) * 512],
                                ones_f[0:1, 0:128],
                                bv_t[:],
                                start=False, stop=True,
                            )
                        nc.gpsimd.memset(vh_t[st][:], 1.0)
                        nc.vector.tensor_copy(
                            vh_t[st]
                            .rearrange("p (h x) -> p h x", x=65)[:, :, 0:64],
                            psv[i][:, half * 512 : (half + 1) * 512].rearrange(
                                "p (h x) -> p h x", x=64
                            ),
                        )

            # late loads (phase D)
            load_big(wp_t, wp_d, DIMC, DIM)
            bp_t = consts.tile([128, 8], dt.float32)
            if use_bp:
                nc.sync.dma_start(bp_t[:], bp_d[:])

            # ====== phase C: attention, PV interleaved into the score loop ======
            with (
                tc.tile_pool(name="pb", bufs=4) as pbp,
                tc.tile_pool(name="attn", bufs=16) as attnp,
                tc.tile_pool(name="scps", bufs=2, space="PSUM") as scps,
                tc.tile_pool(name="pvps", bufs=2, space="PSUM") as pvps,
            ):
                for h in range(NHC):
                    p, j = h // 2, h % 2
                    jj = j * 64
                    pb_t = pbp.tile([128, 8 * L], dt.bfloat16, name=f"pb{h}", tag="pb")
                    load_big(pb_t, pb_d[h * S : (h + 1) * S, :], S, L)
                    po = pvps.tile([65, L], dt.float32, name=f"po{h}", tag="pv")
                    for st in range(8):
                        ps = scps.tile([128, L], dt.float32, name=f"sc{h}_{st}", tag="sc")
                        for lc in range(2):
                            lcs = slice(lc * 512, (lc + 1) * 512)
                            nc.tensor.matmul(
                                ps[:, lcs], id_t[:],
                                pb_t[:, st * 1024 + lc * 512 : st * 1024 + (lc + 1) * 512],
                                start=True, stop=False,
                            )
                            nc.tensor.matmul(
                                ps[:, lcs],
                                kh_t[p][jj : jj + 64, st * 128 : (st + 1) * 128],
                                qh_t[p][jj : jj + 64, lcs],
                                start=False, stop=True,
                                tile_position=(jj, 0),
                            )
                        at = attnp.tile([128, L], dt.bfloat16, name=f"at{h}_{st}", tag="attn")
                        nc.scalar.activation(at[:], ps[:], AF.Exp)
                        mul_eng = nc.vector if st % 2 == 0 else nc.gpsimd
                        mul_eng.tensor_mul(
                            at[:], at[:], mf_t[:, st * 1024 : (st + 1) * 1024]
                        )
                        for lc in range(2):
                            lcs = slice(lc * 512, (lc + 1) * 512)
                            nc.tensor.matmul(
                                po[:, lcs],
                                vh_t[st][:, h * 65 : h * 65 + 65],
                                at[:, lcs],
                                start=(st == 0),
                                stop=(st == 7),
                            )
                    lnr = stage.tile([65, L], dt.float32, name=f"lnr{h}", tag="lnr")
                    nc.scalar.activation(lnr[64:65, :], po[64:65, :], AF.Ln)
                    rec_bf = stage.tile([65, L], dt.bfloat16, name=f"recbf{h}", tag="recbf")
                    nc.scalar.activation(
                        rec_bf[64:65, :], lnr[64:65, :], AF.Exp, scale=-1.0
                    )
                    pr = scps.tile([128, L], dt.float32, name=f"pr{h}", tag="sc")
                    for lc in range(2):
                        lcs = slice(lc * 512, (lc + 1) * 512)
                        nc.tensor.matmul(
                            pr[0:64, lcs],
                            ones_t[64:65, 0:64],
                            rec_bf[64:65, lcs],
                            start=True, stop=True,
                        )
                    pr_sb = stage.tile([64, L], dt.float32, name=f"prsb{h}", tag="prsb")
                    nc.vector.tensor_copy(pr_sb[:], pr[0:64, :])
                    nc.vector.tensor_mul(
                        oT_t[p][j * 64 : (j + 1) * 64, :], po[0:64, :], pr_sb[:]
                    )

            # ================= phase D: output projection =================
            with tc.tile_pool(name="finps", bufs=4, space="PSUM") as finps:
                for ot in range(8):
                    for lc in range(2):
                        lcs = slice(lc * 512, (lc + 1) * 512)
                        pf = finps.tile([128, 512], dt.float32, name=f"pf{ot}_{lc}", tag="fin")
                        for p4 in range(4):
                            nc.tensor.matmul(
                                pf[:],
                                wp_t[:, p4 * 1024 + ot * 128 : p4 * 1024 + (ot + 1) * 128],
                                oT_t[p4][:, lcs],
                                start=(p4 == 0),
                                stop=(p4 == 3),
                            )
                        f_sb = ostage.tile([128, 512], dt.float32, name=f"fsb{ot}_{lc}", tag="fsb")
                        if use_bp:
                            nc.scalar.activation(
                                f_sb[:], pf[:], AF.Identity, bias=bp_t[:, ot : ot + 1]
                            )
                        else:
                            nc.vector.tensor_copy(f_sb[:], pf[:])
                        nc.sync.dma_start(
                            out_d[ot * 128 : (ot + 1) * 128, lcs], f_sb[:]
                        )

    _split_multiwait_instructions(nc)
    _NC_CACHE[key] = nc
    return nc


# ---------------------------------------------------------------- host side
def prep_inputs(inputs):
    """Shard + lay out the full inputs into 8 per-core input maps."""
    q = np.asarray(inputs["q"], np.float32)
    k = np.asarray(inputs["k"], np.float32)
    v = np.asarray(inputs["v"], np.float32)
    attn_mask = np.asarray(inputs["attn_mask"], bool)
    pos_bias = np.asarray(inputs["pos_bias"], np.float32)
    Wq = np.asarray(inputs["Wq"], np.float32)
    Wk = np.asarray(inputs["Wk"], np.float32)
    Wv = np.asarray(inputs["Wv"], np.float32)
    Wp = np.asarray(inputs["Wp"], np.float32)
    bq = np.asarray(inputs["bq"], np.float32)
    bk = np.asarray(inputs["bk"], np.float32)
    bv = np.asarray(inputs["bv"], np.float32)
    bp = np.asarray(inputs["bp"], np.float32)
    is_causal = int(np.asarray(inputs["is_causal"]))

    # effective mask: causal + row-any fix (matches the reference exactly)
    mask = attn_mask
    if is_causal:
        causal = np.tril(np.ones((L, L), bool))
        causal = np.pad(causal, ((0, 0), (S - L, 0)), constant_values=True)
        mask = mask & causal[None]
    row_any = mask.any(axis=-1, keepdims=True)
    mask = np.where(row_any, mask, True)  # (B, L, S)

    ident = np.eye(128, dtype=bf16)
    in_maps = []
    for core in range(8):
        b, hh = core // 2, core % 2
        c0 = hh * DIMC
        h0 = hh * NHC
        wq_c = (Wq[:, c0 : c0 + DIMC] * SCALE).astype(bf16)
        wk_c = Wk[:, c0 : c0 + DIMC].astype(bf16)
        wv_c = Wv[:, c0 : c0 + DIMC].astype(bf16)
        wp_c = Wp[c0 : c0 + DIMC, :].astype(bf16)
        pbT = (
            pos_bias[b, h0 : h0 + NHC]
            .transpose(0, 2, 1)
            .reshape(NHC * S, L)
            .astype(bf16)
        )
        in_maps.append(
            dict(
                qT=q[b].T.astype(bf16),
                kT=k[b].T.astype(bf16),
                vT=v[b].T.astype(bf16),
                wq=np.ascontiguousarray(wq_c),
                wk=np.ascontiguousarray(wk_c),
                wv=np.ascontiguousarray(wv_c),
                wp=np.ascontiguousarray(wp_c),
                pbT=np.ascontiguousarray(pbT),
                mfT=mask[b].T.astype(bf16),
                bq=np.ascontiguousarray((bq[c0 : c0 + DIMC] * SCALE)[None, :]),
                bk=np.ascontiguousarray(bk[c0 : c0 + DIMC][None, :]),
                bv=np.ascontiguousarray(bv[c0 : c0 + DIMC][None, :]),
                bp=(
                    np.ascontiguousarray(bp.reshape(8, 128).T)
                    if hh == 0
                    else np.zeros((128, 8), np.float32)
                ),
                ident=ident,
            )
        )
    return in_maps


def kernel(**inputs):
    global LAST_EXEC_NS
    from concourse.bass_utils import run_bass_kernel_spmd

    nc = build_nc(
        use_bq=bool(np.any(np.asarray(inputs["bq"]))),
        use_bk=bool(np.any(np.asarray(inputs["bk"]))),
        use_bv=bool(np.any(np.asarray(inputs["bv"]))),
        use_bp=bool(np.any(np.asarray(inputs["bp"]))),
    )
    in_maps = prep_inputs(inputs)
    kwargs = {}
    if TRACE and TRACE_DIR:
        kwargs["tmpdir"] = TRACE_DIR
    res = run_bass_kernel_spmd(
        nc, in_maps, core_ids=list(range(8)), trace=TRACE, **kwargs
    )
    LAST_EXEC_NS = res.exec_time_ns
    outs = res.results
    out = np.empty((B, L, DIM), np.float32)
    for b in range(B):
        out[b] = (outs[2 * b]["out"] + outs[2 * b + 1]["out"]).T
    return out


# revision 10
# speedup vs baseline: 1.3584x; 1.3584x over previous
"""Trainium2 Bass kernel for nn_AttentionBlock (B=4, L=S=1024, DIM=1024, NH=16).

Sharding: 8 cores = (batch b = core//2) x (head-half hh = core%2, 8 heads each).
Each core computes its batch's QKV projections restricted to its 512 feature
columns, attention for its 8 heads, and a partial output projection
(Wp row-slice); the host sums the two partials per batch.

Device layout is fully transposed ("T" = features/S on partitions) so no
on-device transposes are needed:
  qhT/khT (feat, L|S) from  Wslice.T @ xT ;  scoresT (S, L) = khT.T-slice @ qhT
  pos_bias enters PSUM via identity-matmul accumulation; mask is applied
  multiplicatively post-exp (host pre-merges causal+row-fix into the mask);
  softmax denominators ride a ones-column appended to V; normalization happens
  on the small (64, L) PV output via a rank-1 PE broadcast of 1/denom.
Compute dtype bf16 (f32 PSUM accumulation), f32 partial outputs.
"""
import contextlib
import ctypes
import sys
import types

import numpy as np
import ml_dtypes

bf16 = ml_dtypes.bfloat16

B, L, S, DIM, NH, DH = 4, 1024, 1024, 1024, 16, 64
NHC = 8           # heads per core
DIMC = 512        # feature columns per core
SCALE = 1.0 / np.sqrt(DH).astype(np.float32)

TRACE = False          # test.py flips this for profiling runs
TRACE_DIR = None
LAST_EXEC_NS = None


# ---------------------------------------------------------------- env setup
def _install_ntff_hook():
    if "antenv.axon_hooks" in sys.modules:
        return
    try:
        lib = ctypes.CDLL("/opt/axon/libaxon_pjrt.so")
        lib.axon_start_nrt_profile.argtypes = [
            ctypes.POINTER(ctypes.c_int64),
            ctypes.c_size_t,
        ]
        lib.axon_start_nrt_profile.restype = ctypes.c_int64
        lib.axon_stop_nrt_profile.argtypes = [ctypes.c_char_p]
        lib.axon_stop_nrt_profile.restype = ctypes.c_int64
    except OSError:
        return

    @contextlib.contextmanager
    def _hook(output_dir, device_ids):
        import jax

        jax.devices()
        if device_ids:
            ids = (ctypes.c_int64 * len(device_ids))(*device_ids)
            rc = lib.axon_start_nrt_profile(ids, len(device_ids))
        else:
            rc = lib.axon_start_nrt_profile(None, 0)
        if rc != 0:
            raise RuntimeError(f"axon_start_nrt_profile rc={rc}")
        try:
            yield
        finally:
            n = lib.axon_stop_nrt_profile(str(output_dir).encode())
            print(f"profile: {n} file(s) written to {output_dir}")

    mod = types.ModuleType("antenv.axon_hooks")
    mod.get_axon_ntff_profile_hook = lambda: _hook
    mod.set_axon_ntff_profile_hook = lambda h: None
    sys.modules["antenv.axon_hooks"] = mod


def _patch_tile_drain():
    from concourse import mybir
    from concourse.tile import TileContext, ScopedClock

    if getattr(TileContext, "_drain_split_patched", False):
        return

    def _drain_and_barrier(self, tick_clock, wait_clock):
        drain_inst = self.nc.sync.drain()
        wait_clock.add_sem_waits(
            drain_inst.ins, ScopedClock({None: tick_clock.global_clock})
        )
        waits = list(drain_inst.ins.sync_info.on_wait)
        if len(waits) > 1:
            drain_inst.ins.sync_info.on_wait = waits[:1]
            for w in waits[1:]:
                nop = self.nc.sync.nop()
                nop.ins.sync_info = mybir.SyncInfo(on_wait=[w], on_update=[])
        self.nc.all_engine_barrier()
        assert self.sems is not None
        popped = self.nc._tile_sem_poison_stack.pop()
        assert popped is self._sem_poison
        self.nc.clear_and_free_semaphores(list(self.sems.allocated().values()))
        self.nc.all_engine_barrier()

    TileContext._drain_and_barrier = _drain_and_barrier
    TileContext._drain_split_patched = True


def _split_multiwait_instructions(nc):
    """This container's walrus rejects >1 sync wait per instruction; hoist
    extras onto same-engine NOPs placed right before the instruction."""
    from concourse import mybir

    n_split = 0
    for fn in nc.m.functions:
        for bb in fn.blocks:
            out = []
            for inst in bb.instructions:
                si = inst.sync_info
                waits = list(si.on_wait) if si is not None else []
                if len(waits) > 1:
                    for w in waits[:-1]:
                        n_split += 1
                        out.append(
                            mybir.InstNoOp(
                                name=f"waitsplit-{n_split}-{inst.name}",
                                engine=inst.engine,
                                bass_nofuse=True,
                                sync_info=mybir.SyncInfo(on_wait=[w], on_update=[]),
                            )
                        )
                    si.on_wait = waits[-1:]
                out.append(inst)
            if n_split:
                bb.instructions = out


# ---------------------------------------------------------------- builder
_NC_CACHE = {}


def build_nc(use_bq=False, use_bk=False, use_bv=False, use_bp=False):
    key = (use_bq, use_bk, use_bv, use_bp)
    if key in _NC_CACHE:
        return _NC_CACHE[key]
    _install_ntff_hook()
    _patch_tile_drain()
    import concourse.bass as bass
    import concourse.tile as tile
    from concourse import mybir

    dt = mybir.dt
    AF = mybir.ActivationFunctionType

    nc = bass.Bass("TRN2", target_bir_lowering=False, debug=False, num_devices=8)

    qT_d = nc.declare_dram_parameter("qT", (DIM, L), dt.bfloat16, isOutput=False)
    kT_d = nc.declare_dram_parameter("kT", (DIM, S), dt.bfloat16, isOutput=False)
    vT_d = nc.declare_dram_parameter("vT", (DIM, S), dt.bfloat16, isOutput=False)
    wq_d = nc.declare_dram_parameter("wq", (DIM, DIMC), dt.bfloat16, isOutput=False)
    wk_d = nc.declare_dram_parameter("wk", (DIM, DIMC), dt.bfloat16, isOutput=False)
    wv_d = nc.declare_dram_parameter("wv", (DIM, DIMC), dt.bfloat16, isOutput=False)
    wp_d = nc.declare_dram_parameter("wp", (DIMC, DIM), dt.bfloat16, isOutput=False)
    pb_d = nc.declare_dram_parameter("pbT", (NHC * S, L), dt.bfloat16, isOutput=False)
    mf_d = nc.declare_dram_parameter("mfT", (S, L), dt.bfloat16, isOutput=False)
    bq_d = nc.declare_dram_parameter("bq", (1, DIMC), dt.float32, isOutput=False)
    bk_d = nc.declare_dram_parameter("bk", (1, DIMC), dt.float32, isOutput=False)
    bv_d = nc.declare_dram_parameter("bv", (1, DIMC), dt.float32, isOutput=False)
    bp_d = nc.declare_dram_parameter("bp", (128, 8), dt.float32, isOutput=False)
    id_d = nc.declare_dram_parameter("ident", (128, 128), dt.bfloat16, isOutput=False)
    out_d = nc.declare_dram_parameter("out", (DIM, L), dt.float32, isOutput=True)

    with tile.TileContext(nc) as tc:
        with (
            tc.tile_pool(name="consts", bufs=1) as consts,
            tc.tile_pool(name="w", bufs=1) as wpool,
            tc.tile_pool(name="mf", bufs=1) as mfpool,
            tc.tile_pool(name="heads", bufs=1) as heads,
            tc.tile_pool(name="stage", bufs=1) as stage,
            tc.tile_pool(name="ostage", bufs=4) as ostage,
        ):
            ones_t = consts.tile([128, 64], dt.bfloat16)
            nc.gpsimd.memset(ones_t[:], 1.0)
            if use_bq:
                bq_t = consts.tile([1, DIMC], dt.float32)
                nc.sync.dma_start(bq_t[:], bq_d[:])
            if use_bk:
                bk_t = consts.tile([1, DIMC], dt.float32)
                nc.sync.dma_start(bk_t[:], bk_d[:])
            if use_bv:
                bv_t = consts.tile([1, DIMC], dt.float32)
                nc.sync.dma_start(bv_t[:], bv_d[:])
                ones_f = consts.tile([1, 128], dt.float32)
                nc.gpsimd.memset(ones_f[:], 1.0)
            if use_bq or use_bk:
                ones_r = consts.tile([1, 512], dt.float32)
                nc.gpsimd.memset(ones_r[:], 1.0)

            # big consolidated weight tiles: w*[:, dt*512 + p*128 + ...] = W[dt*128+p-row, col]
            wq_t = wpool.tile([128, 8 * DIMC], dt.bfloat16, name="wqb", tag="wqb")
            wk_t = wpool.tile([128, 8 * DIMC], dt.bfloat16, name="wkb", tag="wkb")
            wv_t = wpool.tile([128, 8 * DIMC], dt.bfloat16, name="wvb", tag="wvb")
            wp_t = wpool.tile([128, 4 * DIM], dt.bfloat16, name="wpb", tag="wpb")
            mf_t = mfpool.tile([128, 8 * L], dt.bfloat16, name="mfb", tag="mfb")

            qh_t = [heads.tile([128, L], dt.bfloat16, name=f"qh{i}", tag=f"qh{i}") for i in range(4)]
            kh_t = [heads.tile([128, S], dt.bfloat16, name=f"kh{i}", tag=f"kh{i}") for i in range(4)]
            vh_t = [heads.tile([128, NHC * 65], dt.bfloat16, name=f"vh{i}", tag=f"vh{i}") for i in range(8)]
            oT_t = [heads.tile([128, L], dt.bfloat16, name=f"oT{i}", tag=f"oT{i}") for i in range(4)]

            def load_big(tile_ap, dram, rows, cols, chunks=1):
                # tile[:, a*cols + c] = dram[a*128 + p, c]
                n_a = rows // 128
                a_per = n_a // chunks
                for ch in range(chunks):
                    nc.sync.dma_start(
                        tile_ap[:, ch * a_per * cols : (ch + 1) * a_per * cols]
                        .rearrange("p (a c) -> p a c", c=cols),
                        dram[ch * a_per * 128 : (ch + 1) * a_per * 128, :]
                        .rearrange("(a p) c -> p a c", p=128),
                    )

            # ================= phase B: projections =================
            with (
                tc.tile_pool(name="xT", bufs=1) as xTp,
                tc.tile_pool(name="projps", bufs=4, space="PSUM") as pps,
            ):
                xb = {}
                for nm in ("q", "k", "v"):
                    xb[nm] = xTp.tile([128, 8 * 1024], dt.bfloat16, name=f"{nm}Tb", tag=f"{nm}Tb")

                load_big(wq_t, wq_d, DIM, DIMC)
                load_big(xb["q"], qT_d, DIM, L, chunks=2)
                load_big(wk_t, wk_d, DIM, DIMC)
                load_big(xb["k"], kT_d, DIM, S, chunks=2)
                id_t = consts.tile([128, 128], dt.bfloat16)
                nc.sync.dma_start(id_t[:], id_d[:])
                load_big(mf_t, mf_d, S, L)

                # Q+K projections interleaved per head-pair so pair 0 finishes
                # early and attention can start while pairs 1-3 project
                for p in range(4):
                    psq = pps.tile([128, 1024], dt.float32, name=f"psq{p}", tag="ps")
                    psk = pps.tile([128, 1024], dt.float32, name=f"psk{p}", tag="ps")
                    for dtile in range(8):
                        for ps, w_t, x_t in ((psq, wq_t, xb["q"]), (psk, wk_t, xb["k"])):
                            for lc in range(2):
                                nc.tensor.matmul(
                                    ps[:, lc * 512 : (lc + 1) * 512],
                                    w_t[:, dtile * 512 + p * 128 : dtile * 512 + (p + 1) * 128],
                                    x_t[:, dtile * 1024 + lc * 512 : dtile * 1024 + (lc + 1) * 512],
                                    start=(dtile == 0),
                                    stop=(dtile == 7)
                                    and not (use_bq if w_t is wq_t else use_bk),
                                )
                    if use_bq:
                        for lc in range(2):
                            nc.tensor.matmul(
                                psq[:, lc * 512 : (lc + 1) * 512],
                                bq_t[0:1, p * 128 : (p + 1) * 128],
                                ones_r[0:1, 0:512],
                                start=False, stop=True,
                            )
                    if use_bk:
                        for lc in range(2):
                            nc.tensor.matmul(
                                psk[:, lc * 512 : (lc + 1) * 512],
                                bk_t[0:1, p * 128 : (p + 1) * 128],
                                ones_r[0:1, 0:512],
                                start=False, stop=True,
                            )
                    nc.vector.tensor_copy(qh_t[p][:], psq[:])
                    nc.vector.tensor_copy(kh_t[p][:], psk[:])

                # V: normal layout (S on partitions), ones column interleaved
                load_big(xb["v"], vT_d, DIM, S, chunks=2)
                load_big(wv_t, wv_d, DIM, DIMC)
                psv = [pps.tile([128, 1024], dt.float32, name=f"psv{i}", tag="ps") for i in range(4)]
                for dtile in range(8):
                    for i in range(4):
                        for half in range(2):
                            st = 2 * i + half
                            nc.tensor.matmul(
                                psv[i][:, half * 512 : (half + 1) * 512],
                                xb["v"][:, dtile * 1024 + st * 128 : dtile * 1024 + (st + 1) * 128],
                                wv_t[:, dtile * 512 : (dtile + 1) * 512],
                                start=(dtile == 0),
                                stop=(dtile == 7) and not use_bv,
                            )
                for i in range(4):
                    for half in range(2):
                        st = 2 * i + half
                        if use_bv:
                            nc.tensor.matmul(
                                psv[i][:, half * 512 : (half + 1) * 512],
                                ones_f[0:1, 0:128],
                                bv_t[:],
                                start=False, stop=True,
                            )
                        nc.gpsimd.memset(vh_t[st][:], 1.0)
                        nc.vector.tensor_copy(
                            vh_t[st]
                            .rearrange("p (h x) -> p h x", x=65)[:, :, 0:64],
                            psv[i][:, half * 512 : (half + 1) * 512].rearrange(
                                "p (h x) -> p h x", x=64
                            ),
                        )

            # late loads (phase D)
            load_big(wp_t, wp_d, DIMC, DIM)
            bp_t = consts.tile([128, 8], dt.float32)
            if use_bp:
                nc.sync.dma_start(bp_t[:], bp_d[:])

            # ====== phase C: attention; PV + normalize software-pipelined ======
            with (
                tc.tile_pool(name="pb", bufs=4) as pbp,
                tc.tile_pool(name="attn", bufs=16) as attnp,
                tc.tile_pool(name="scps", bufs=2, space="PSUM") as scps,
                tc.tile_pool(name="pvps", bufs=2, space="PSUM") as pvps,
            ):
                LAG = 3       # PV(unit i) emitted after scores(unit i+LAG)
                NLAG = 3      # extra units before a head's normalize is emitted
                units = [(h, st) for h in range(NHC) for st in range(8)]
                ats = {}
                pos = {}

                def emit_scores(i):
                    h, st = units[i]
                    p, j = h // 2, h % 2
                    jj = j * 64
                    if st == 0:
                        pb_t = pbp.tile([128, 8 * L], dt.bfloat16, name=f"pb{h}", tag="pb")
                        load_big(pb_t, pb_d[h * S : (h + 1) * S, :], S, L)
                        pos[h] = (
                            pvps.tile([65, L], dt.float32, name=f"po{h}", tag="pv"),
                            pb_t,
                        )
                    pb_t = pos[h][1]
                    ps = scps.tile([128, L], dt.float32, name=f"sc{h}_{st}", tag="sc")
                    for lc in range(2):
                        lcs = slice(lc * 512, (lc + 1) * 512)
                        nc.tensor.matmul(
                            ps[:, lcs], id_t[:],
                            pb_t[:, st * 1024 + lc * 512 : st * 1024 + (lc + 1) * 512],
                            start=True, stop=False,
                        )
                        nc.tensor.matmul(
                            ps[:, lcs],
                            kh_t[p][jj : jj + 64, st * 128 : (st + 1) * 128],
                            qh_t[p][jj : jj + 64, lcs],
                            start=False, stop=True,
                            tile_position=(jj, 0),
                        )
                    at = attnp.tile([128, L], dt.bfloat16, name=f"at{h}_{st}", tag="attn")
                    nc.scalar.activation(at[:], ps[:], AF.Exp)
                    nc.vector.tensor_mul(
                        at[:], at[:], mf_t[:, st * 1024 : (st + 1) * 1024]
                    )
                    ats[(h, st)] = at

                def emit_pv(i):
                    h, st = units[i]
                    po = pos[h][0]
                    at = ats.pop((h, st))
                    for lc in range(2):
                        lcs = slice(lc * 512, (lc + 1) * 512)
                        nc.tensor.matmul(
                            po[:, lcs],
                            vh_t[st][:, h * 65 : h * 65 + 65],
                            at[:, lcs],
                            start=(st == 0),
                            stop=(st == 7),
                        )

                def emit_norm(h):
                    p, j = h // 2, h % 2
                    po = pos.pop(h)[0]
                    lnr = stage.tile([65, L], dt.float32, name=f"lnr{h}", tag="lnr")
                    nc.scalar.activation(lnr[64:65, :], po[64:65, :], AF.Ln)
                    rec_bf = stage.tile([65, L], dt.bfloat16, name=f"recbf{h}", tag="recbf")
                    nc.scalar.activation(
                        rec_bf[64:65, :], lnr[64:65, :], AF.Exp, scale=-1.0
                    )
                    pr = scps.tile([128, L], dt.float32, name=f"pr{h}", tag="sc")
                    for lc in range(2):
                        lcs = slice(lc * 512, (lc + 1) * 512)
                        nc.tensor.matmul(
                            pr[0:64, lcs],
                            ones_t[64:65, 0:64],
                            rec_bf[64:65, lcs],
                            start=True, stop=True,
                        )
                    pr_sb = stage.tile([64, L], dt.float32, name=f"prsb{h}", tag="prsb")
                    nc.vector.tensor_copy(pr_sb[:], pr[0:64, :])
                    nc.vector.tensor_mul(
                        oT_t[p][j * 64 : (j + 1) * 64, :], po[0:64, :], pr_sb[:]
                    )

                norm_due = {}  # emission index -> head
                n_units = len(units)
                for i in range(n_units + LAG + NLAG + 1):
                    if i in norm_due:
                        emit_norm(norm_due.pop(i))
                    if i < n_units:
                        emit_scores(i)
                    ipv = i - LAG
                    if 0 <= ipv < n_units:
                        emit_pv(ipv)
                        h, st = units[ipv]
                        if st == 7:
                            norm_due[i + NLAG] = h
                assert not norm_due and not pos and not ats

            # ================= phase D: output projection =================
            with tc.tile_pool(name="finps", bufs=4, space="PSUM") as finps:
                for ot in range(8):
                    for lc in range(2):
                        lcs = slice(lc * 512, (lc + 1) * 512)
                        pf = finps.tile([128, 512], dt.float32, name=f"pf{ot}_{lc}", tag="fin")
                        for p4 in range(4):
                            nc.tensor.matmul(
                                pf[:],
                                wp_t[:, p4 * 1024 + ot * 128 : p4 * 1024 + (ot + 1) * 128],
                                oT_t[p4][:, lcs],
                                start=(p4 == 0),
                                stop=(p4 == 3),
                            )
                        f_sb = ostage.tile([128, 512], dt.float32, name=f"fsb{ot}_{lc}", tag="fsb")
                        if use_bp:
                            nc.scalar.activation(
                                f_sb[:], pf[:], AF.Identity, bias=bp_t[:, ot : ot + 1]
                            )
                        else:
                            nc.vector.tensor_copy(f_sb[:], pf[:])
                        nc.sync.dma_start(
                            out_d[ot * 128 : (ot + 1) * 128, lcs], f_sb[:]
                        )

    _split_multiwait_instructions(nc)
    _NC_CACHE[key] = nc
    return nc


# ---------------------------------------------------------------- host side
def prep_inputs(inputs):
    """Shard + lay out the full inputs into 8 per-core input maps."""
    q = np.asarray(inputs["q"], np.float32)
    k = np.asarray(inputs["k"], np.float32)
    v = np.asarray(inputs["v"], np.float32)
    attn_mask = np.asarray(inputs["attn_mask"], bool)
    pos_bias = np.asarray(inputs["pos_bias"], np.float32)
    Wq = np.asarray(inputs["Wq"], np.float32)
    Wk = np.asarray(inputs["Wk"], np.float32)
    Wv = np.asarray(inputs["Wv"], np.float32)
    Wp = np.asarray(inputs["Wp"], np.float32)
    bq = np.asarray(inputs["bq"], np.float32)
    bk = np.asarray(inputs["bk"], np.float32)
    bv = np.asarray(inputs["bv"], np.float32)
    bp = np.asarray(inputs["bp"], np.float32)
    is_causal = int(np.asarray(inputs["is_causal"]))

    # effective mask: causal + row-any fix (matches the reference exactly)
    mask = attn_mask
    if is_causal:
        causal = np.tril(np.ones((L, L), bool))
        causal = np.pad(causal, ((0, 0), (S - L, 0)), constant_values=True)
        mask = mask & causal[None]
    row_any = mask.any(axis=-1, keepdims=True)
    mask = np.where(row_any, mask, True)  # (B, L, S)

    ident = np.eye(128, dtype=bf16)
    in_maps = []
    for core in range(8):
        b, hh = core // 2, core % 2
        c0 = hh * DIMC
        h0 = hh * NHC
        wq_c = (Wq[:, c0 : c0 + DIMC] * SCALE).astype(bf16)
        wk_c = Wk[:, c0 : c0 + DIMC].astype(bf16)
        wv_c = Wv[:, c0 : c0 + DIMC].astype(bf16)
        wp_c = Wp[c0 : c0 + DIMC, :].astype(bf16)
        pbT = (
            pos_bias[b, h0 : h0 + NHC]
            .transpose(0, 2, 1)
            .reshape(NHC * S, L)
            .astype(bf16)
        )
        in_maps.append(
            dict(
                qT=q[b].T.astype(bf16),
                kT=k[b].T.astype(bf16),
                vT=v[b].T.astype(bf16),
                wq=np.ascontiguousarray(wq_c),
                wk=np.ascontiguousarray(wk_c),
                wv=np.ascontiguousarray(wv_c),
                wp=np.ascontiguousarray(wp_c),
                pbT=np.ascontiguousarray(pbT),
                mfT=mask[b].T.astype(bf16),
                bq=np.ascontiguousarray((bq[c0 : c0 + DIMC] * SCALE)[None, :]),
                bk=np.ascontiguousarray(bk[c0 : c0 + DIMC][None, :]),
                bv=np.ascontiguousarray(bv[c0 : c0 + DIMC][None, :]),
                bp=(
                    np.ascontiguousarray(bp.reshape(8, 128).T)
                    if hh == 0
                    else np.zeros((128, 8), np.float32)
                ),
                ident=ident,
            )
        )
    return in_maps


def kernel(**inputs):
    global LAST_EXEC_NS
    from concourse.bass_utils import run_bass_kernel_spmd

    nc = build_nc(
        use_bq=bool(np.any(np.asarray(inputs["bq"]))),
        use_bk=bool(np.any(np.asarray(inputs["bk"]))),
        use_bv=bool(np.any(np.asarray(inputs["bv"]))),
        use_bp=bool(np.any(np.asarray(inputs["bp"]))),
    )
    in_maps = prep_inputs(inputs)
    kwargs = {}
    if TRACE and TRACE_DIR:
        kwargs["tmpdir"] = TRACE_DIR
    res = run_bass_kernel_spmd(
        nc, in_maps, core_ids=list(range(8)), trace=TRACE, **kwargs
    )
    LAST_EXEC_NS = res.exec_time_ns
    outs = res.results
    out = np.empty((B, L, DIM), np.float32)
    for b in range(B):
        out[b] = (outs[2 * b]["out"] + outs[2 * b + 1]["out"]).T
    return out


# revision 11
# speedup vs baseline: 1.4265x; 1.0502x over previous
"""Trainium2 Bass kernel for nn_AttentionBlock (B=4, L=S=1024, DIM=1024, NH=16).

Sharding: 8 cores = (batch b = core//2) x (head-half hh = core%2, 8 heads each).
Each core computes its batch's QKV projections restricted to its 512 feature
columns, attention for its 8 heads, and a partial output projection
(Wp row-slice); the host sums the two partials per batch.

Device layout is fully transposed ("T" = features/S on partitions) so no
on-device transposes are needed:
  qhT/khT (feat, L|S) from  Wslice.T @ xT ;  scoresT (S, L) = khT.T-slice @ qhT
  pos_bias enters PSUM via identity-matmul accumulation; mask is applied
  multiplicatively post-exp (host pre-merges causal+row-fix into the mask);
  softmax denominators ride a ones-column appended to V; normalization happens
  on the small (64, L) PV output via a rank-1 PE broadcast of 1/denom.
Compute dtype bf16 (f32 PSUM accumulation), f32 partial outputs.
"""
import contextlib
import ctypes
import sys
import types

import numpy as np
import ml_dtypes

bf16 = ml_dtypes.bfloat16

B, L, S, DIM, NH, DH = 4, 1024, 1024, 1024, 16, 64
NHC = 8           # heads per core
DIMC = 512        # feature columns per core
SCALE = 1.0 / np.sqrt(DH).astype(np.float32)

TRACE = False          # test.py flips this for profiling runs
TRACE_DIR = None
LAST_EXEC_NS = None


# ---------------------------------------------------------------- env setup
def _install_ntff_hook():
    if "antenv.axon_hooks" in sys.modules:
        return
    try:
        lib = ctypes.CDLL("/opt/axon/libaxon_pjrt.so")
        lib.axon_start_nrt_profile.argtypes = [
            ctypes.POINTER(ctypes.c_int64),
            ctypes.c_size_t,
        ]
        lib.axon_start_nrt_profile.restype = ctypes.c_int64
        lib.axon_stop_nrt_profile.argtypes = [ctypes.c_char_p]
        lib.axon_stop_nrt_profile.restype = ctypes.c_int64
    except OSError:
        return

    @contextlib.contextmanager
    def _hook(output_dir, device_ids):
        import jax

        jax.devices()
        if device_ids:
            ids = (ctypes.c_int64 * len(device_ids))(*device_ids)
            rc = lib.axon_start_nrt_profile(ids, len(device_ids))
        else:
            rc = lib.axon_start_nrt_profile(None, 0)
        if rc != 0:
            raise RuntimeError(f"axon_start_nrt_profile rc={rc}")
        try:
            yield
        finally:
            n = lib.axon_stop_nrt_profile(str(output_dir).encode())
            print(f"profile: {n} file(s) written to {output_dir}")

    mod = types.ModuleType("antenv.axon_hooks")
    mod.get_axon_ntff_profile_hook = lambda: _hook
    mod.set_axon_ntff_profile_hook = lambda h: None
    sys.modules["antenv.axon_hooks"] = mod


def _patch_tile_drain():
    from concourse import mybir
    from concourse.tile import TileContext, ScopedClock

    if getattr(TileContext, "_drain_split_patched", False):
        return

    def _drain_and_barrier(self, tick_clock, wait_clock):
        drain_inst = self.nc.sync.drain()
        wait_clock.add_sem_waits(
            drain_inst.ins, ScopedClock({None: tick_clock.global_clock})
        )
        waits = list(drain_inst.ins.sync_info.on_wait)
        if len(waits) > 1:
            drain_inst.ins.sync_info.on_wait = waits[:1]
            for w in waits[1:]:
                nop = self.nc.sync.nop()
                nop.ins.sync_info = mybir.SyncInfo(on_wait=[w], on_update=[])
        self.nc.all_engine_barrier()
        assert self.sems is not None
        popped = self.nc._tile_sem_poison_stack.pop()
        assert popped is self._sem_poison
        self.nc.clear_and_free_semaphores(list(self.sems.allocated().values()))
        self.nc.all_engine_barrier()

    TileContext._drain_and_barrier = _drain_and_barrier
    TileContext._drain_split_patched = True


def _split_multiwait_instructions(nc):
    """This container's walrus rejects >1 sync wait per instruction; hoist
    extras onto same-engine NOPs placed right before the instruction."""
    from concourse import mybir

    n_split = 0
    for fn in nc.m.functions:
        for bb in fn.blocks:
            out = []
            for inst in bb.instructions:
                si = inst.sync_info
                waits = list(si.on_wait) if si is not None else []
                if len(waits) > 1:
                    for w in waits[:-1]:
                        n_split += 1
                        out.append(
                            mybir.InstNoOp(
                                name=f"waitsplit-{n_split}-{inst.name}",
                                engine=inst.engine,
                                bass_nofuse=True,
                                sync_info=mybir.SyncInfo(on_wait=[w], on_update=[]),
                            )
                        )
                    si.on_wait = waits[-1:]
                out.append(inst)
            if n_split:
                bb.instructions = out


# ---------------------------------------------------------------- builder
_NC_CACHE = {}


def build_nc(use_bq=False, use_bk=False, use_bv=False, use_bp=False):
    key = (use_bq, use_bk, use_bv, use_bp)
    if key in _NC_CACHE:
        return _NC_CACHE[key]
    _install_ntff_hook()
    _patch_tile_drain()
    import concourse.bass as bass
    import concourse.tile as tile
    from concourse import mybir

    dt = mybir.dt
    AF = mybir.ActivationFunctionType

    nc = bass.Bass("TRN2", target_bir_lowering=False, debug=False, num_devices=8)

    qT_d = nc.declare_dram_parameter("qT", (DIM, L), dt.bfloat16, isOutput=False)
    kT_d = nc.declare_dram_parameter("kT", (DIM, S), dt.bfloat16, isOutput=False)
    vT_d = nc.declare_dram_parameter("vT", (DIM, S), dt.bfloat16, isOutput=False)
    wq_d = nc.declare_dram_parameter("wq", (DIM, DIMC), dt.bfloat16, isOutput=False)
    wk_d = nc.declare_dram_parameter("wk", (DIM, DIMC), dt.bfloat16, isOutput=False)
    wv_d = nc.declare_dram_parameter("wv", (DIM, DIMC), dt.bfloat16, isOutput=False)
    wp_d = nc.declare_dram_parameter("wp", (DIMC, DIM), dt.bfloat16, isOutput=False)
    pb_d = nc.declare_dram_parameter("pbT", (NHC * S, L), dt.bfloat16, isOutput=False)
    mf_d = nc.declare_dram_parameter("mfT", (S, L), dt.bfloat16, isOutput=False)
    bq_d = nc.declare_dram_parameter("bq", (1, DIMC), dt.float32, isOutput=False)
    bk_d = nc.declare_dram_parameter("bk", (1, DIMC), dt.float32, isOutput=False)
    bv_d = nc.declare_dram_parameter("bv", (1, DIMC), dt.float32, isOutput=False)
    bp_d = nc.declare_dram_parameter("bp", (128, 8), dt.float32, isOutput=False)
    id_d = nc.declare_dram_parameter("ident", (128, 128), dt.bfloat16, isOutput=False)
    out_d = nc.declare_dram_parameter("out", (DIM, L), dt.float32, isOutput=True)

    with tile.TileContext(nc) as tc:
        with (
            tc.tile_pool(name="consts", bufs=1) as consts,
            tc.tile_pool(name="w", bufs=1) as wpool,
            tc.tile_pool(name="mf", bufs=1) as mfpool,
            tc.tile_pool(name="heads", bufs=1) as heads,
            tc.tile_pool(name="stage", bufs=1) as stage,
            tc.tile_pool(name="ostage", bufs=4) as ostage,
        ):
            ones_t = consts.tile([128, 64], dt.bfloat16)
            nc.gpsimd.memset(ones_t[:], 1.0)
            if use_bq:
                bq_t = consts.tile([1, DIMC], dt.float32)
                nc.sync.dma_start(bq_t[:], bq_d[:])
            if use_bk:
                bk_t = consts.tile([1, DIMC], dt.float32)
                nc.sync.dma_start(bk_t[:], bk_d[:])
            if use_bv:
                bv_t = consts.tile([1, DIMC], dt.float32)
                nc.sync.dma_start(bv_t[:], bv_d[:])
                ones_f = consts.tile([1, 128], dt.float32)
                nc.gpsimd.memset(ones_f[:], 1.0)
            if use_bq or use_bk:
                ones_r = consts.tile([1, 512], dt.float32)
                nc.gpsimd.memset(ones_r[:], 1.0)

            # big consolidated weight tiles: w*[:, dt*512 + p*128 + ...] = W[dt*128+p-row, col]
            wq_t = wpool.tile([128, 8 * DIMC], dt.bfloat16, name="wqb", tag="wqb")
            wk_t = wpool.tile([128, 8 * DIMC], dt.bfloat16, name="wkb", tag="wkb")
            wv_t = wpool.tile([128, 8 * DIMC], dt.bfloat16, name="wvb", tag="wvb")
            wp_t = wpool.tile([128, 4 * DIM], dt.bfloat16, name="wpb", tag="wpb")
            mf_t = mfpool.tile([128, 8 * L], dt.bfloat16, name="mfb", tag="mfb")

            qh_t = [heads.tile([128, L], dt.bfloat16, name=f"qh{i}", tag=f"qh{i}") for i in range(4)]
            kh_t = [heads.tile([128, S], dt.bfloat16, name=f"kh{i}", tag=f"kh{i}") for i in range(4)]
            vh_t = [heads.tile([128, NHC * 65], dt.bfloat16, name=f"vh{i}", tag=f"vh{i}") for i in range(8)]
            oT_t = [heads.tile([128, L], dt.bfloat16, name=f"oT{i}", tag=f"oT{i}") for i in range(4)]

            def load_big(tile_ap, dram, rows, cols, chunks=1):
                # tile[:, a*cols + c] = dram[a*128 + p, c]
                n_a = rows // 128
                a_per = n_a // chunks
                for ch in range(chunks):
                    nc.sync.dma_start(
                        tile_ap[:, ch * a_per * cols : (ch + 1) * a_per * cols]
                        .rearrange("p (a c) -> p a c", c=cols),
                        dram[ch * a_per * 128 : (ch + 1) * a_per * 128, :]
                        .rearrange("(a p) c -> p a c", p=128),
                    )

            # ================= phase B: projections =================
            with (
                tc.tile_pool(name="xT", bufs=1) as xTp,
                tc.tile_pool(name="projps", bufs=4, space="PSUM") as pps,
            ):
                xb = {}
                for nm in ("q", "k", "v"):
                    xb[nm] = xTp.tile([128, 8 * 1024], dt.bfloat16, name=f"{nm}Tb", tag=f"{nm}Tb")

                load_big(wq_t, wq_d, DIM, DIMC)
                load_big(xb["q"], qT_d, DIM, L, chunks=2)
                load_big(wk_t, wk_d, DIM, DIMC)
                load_big(xb["k"], kT_d, DIM, S, chunks=2)
                id_t = consts.tile([128, 128], dt.bfloat16)
                nc.sync.dma_start(id_t[:], id_d[:])
                load_big(mf_t, mf_d, S, L)

                # Q+K projections interleaved per head-pair so pair 0 finishes
                # early and attention can start while pairs 1-3 project
                for p in range(4):
                    psq = pps.tile([128, 1024], dt.float32, name=f"psq{p}", tag="ps")
                    psk = pps.tile([128, 1024], dt.float32, name=f"psk{p}", tag="ps")
                    for dtile in range(8):
                        for ps, w_t, x_t in ((psq, wq_t, xb["q"]), (psk, wk_t, xb["k"])):
                            for lc in range(2):
                                nc.tensor.matmul(
                                    ps[:, lc * 512 : (lc + 1) * 512],
                                    w_t[:, dtile * 512 + p * 128 : dtile * 512 + (p + 1) * 128],
                                    x_t[:, dtile * 1024 + lc * 512 : dtile * 1024 + (lc + 1) * 512],
                                    start=(dtile == 0),
                                    stop=(dtile == 7)
                                    and not (use_bq if w_t is wq_t else use_bk),
                                )
                    if use_bq:
                        for lc in range(2):
                            nc.tensor.matmul(
                                psq[:, lc * 512 : (lc + 1) * 512],
                                bq_t[0:1, p * 128 : (p + 1) * 128],
                                ones_r[0:1, 0:512],
                                start=False, stop=True,
                            )
                    if use_bk:
                        for lc in range(2):
                            nc.tensor.matmul(
                                psk[:, lc * 512 : (lc + 1) * 512],
                                bk_t[0:1, p * 128 : (p + 1) * 128],
                                ones_r[0:1, 0:512],
                                start=False, stop=True,
                            )
                    nc.vector.tensor_copy(qh_t[p][:], psq[:])
                    nc.vector.tensor_copy(kh_t[p][:], psk[:])

                # V: normal layout (S on partitions), ones column interleaved
                load_big(xb["v"], vT_d, DIM, S, chunks=2)
                load_big(wv_t, wv_d, DIM, DIMC)
                psv = [pps.tile([128, 1024], dt.float32, name=f"psv{i}", tag="ps") for i in range(4)]
                for dtile in range(8):
                    for i in range(4):
                        for half in range(2):
                            st = 2 * i + half
                            nc.tensor.matmul(
                                psv[i][:, half * 512 : (half + 1) * 512],
                                xb["v"][:, dtile * 1024 + st * 128 : dtile * 1024 + (st + 1) * 128],
                                wv_t[:, dtile * 512 : (dtile + 1) * 512],
                                start=(dtile == 0),
                                stop=(dtile == 7) and not use_bv,
                            )
                for i in range(4):
                    for half in range(2):
                        st = 2 * i + half
                        if use_bv:
                            nc.tensor.matmul(
                                psv[i][:, half * 512 : (half + 1) * 512],
                                ones_f[0:1, 0:128],
                                bv_t[:],
                                start=False, stop=True,
                            )
                        nc.gpsimd.memset(vh_t[st][:], 1.0)
                        nc.vector.tensor_copy(
                            vh_t[st]
                            .rearrange("p (h x) -> p h x", x=65)[:, :, 0:64],
                            psv[i][:, half * 512 : (half + 1) * 512].rearrange(
                                "p (h x) -> p h x", x=64
                            ),
                        )

            # late loads (phase D)
            load_big(wp_t, wp_d, DIMC, DIM)
            bp_t = consts.tile([128, 8], dt.float32)
            if use_bp:
                nc.sync.dma_start(bp_t[:], bp_d[:])

            # ====== phase C: attention; pair-adjacent scores, lc-granular pipeline ======
            with (
                tc.tile_pool(name="pb", bufs=4) as pbp,
                tc.tile_pool(name="attn", bufs=24) as attnp,
                tc.tile_pool(name="scps", bufs=4, space="PSUM") as scps,
                tc.tile_pool(name="pvps", bufs=2, space="PSUM") as pvps,
            ):
                LAG = 3
                NLAG = 2
                units = [
                    (p, st, lc) for p in range(4) for st in range(8) for lc in range(2)
                ]
                ats = {}
                pos = {}
                pbs = {}

                def emit_scores(i):
                    p, st, lc = units[i]
                    lcs = slice(lc * 512, (lc + 1) * 512)
                    if st == 0 and lc == 0:
                        for j in range(2):
                            h = 2 * p + j
                            pb_t = pbp.tile([128, 8 * L], dt.bfloat16, name=f"pb{h}", tag="pb")
                            load_big(pb_t, pb_d[h * S : (h + 1) * S, :], S, L)
                            pbs[h] = pb_t
                            pos[h] = pvps.tile([65, L], dt.float32, name=f"po{h}", tag="pv")
                    pss = []
                    for j in range(2):
                        h = 2 * p + j
                        ps = scps.tile([128, 512], dt.float32, name=f"sc{h}_{st}_{lc}", tag="sc")
                        nc.tensor.matmul(
                            ps[:], id_t[:],
                            pbs[h][:, st * 1024 + lc * 512 : st * 1024 + (lc + 1) * 512],
                            start=True, stop=False,
                        )
                        pss.append(ps)
                    for j in range(2):
                        h = 2 * p + j
                        jj = j * 64
                        nc.tensor.matmul(
                            pss[j][:],
                            kh_t[p][jj : jj + 64, st * 128 : (st + 1) * 128],
                            qh_t[p][jj : jj + 64, lcs],
                            start=False, stop=True,
                            tile_position=(jj, 0),
                        )
                    for j in range(2):
                        h = 2 * p + j
                        at = attnp.tile([128, 512], dt.bfloat16, name=f"at{h}_{st}_{lc}", tag="attn")
                        nc.scalar.activation(at[:], pss[j][:], AF.Exp)
                        nc.vector.tensor_mul(
                            at[:], at[:], mf_t[:, st * 1024 + lc * 512 : st * 1024 + (lc + 1) * 512]
                        )
                        ats[(h, st, lc)] = at

                def emit_pv(i):
                    p, st, lc = units[i]
                    lcs = slice(lc * 512, (lc + 1) * 512)
                    for j in range(2):
                        h = 2 * p + j
                        at = ats.pop((h, st, lc))
                        nc.tensor.matmul(
                            pos[h][:, lcs],
                            vh_t[st][:, h * 65 : h * 65 + 65],
                            at[:],
                            start=(st == 0),
                            stop=(st == 7),
                        )

                def emit_norm(h):
                    p, j = h // 2, h % 2
                    po = pos.pop(h)
                    pbs.pop(h, None)
                    lnr = stage.tile([65, L], dt.float32, name=f"lnr{h}", tag="lnr")
                    nc.scalar.activation(lnr[64:65, :], po[64:65, :], AF.Ln)
                    rec_bf = stage.tile([65, L], dt.bfloat16, name=f"recbf{h}", tag="recbf")
                    nc.scalar.activation(
                        rec_bf[64:65, :], lnr[64:65, :], AF.Exp, scale=-1.0
                    )
                    pr = scps.tile([128, 512], dt.float32, name=f"pr{h}a", tag="sc")
                    pr2 = scps.tile([128, 512], dt.float32, name=f"pr{h}b", tag="sc")
                    for lc, prt in ((0, pr), (1, pr2)):
                        nc.tensor.matmul(
                            prt[0:64, :],
                            ones_t[64:65, 0:64],
                            rec_bf[64:65, lc * 512 : (lc + 1) * 512],
                            start=True, stop=True,
                        )
                    pr_sb = stage.tile([64, L], dt.float32, name=f"prsb{h}", tag="prsb")
                    nc.vector.tensor_copy(pr_sb[:, 0:512], pr[0:64, :])
                    nc.vector.tensor_copy(pr_sb[:, 512:1024], pr2[0:64, :])
                    nc.vector.tensor_mul(
                        oT_t[p][j * 64 : (j + 1) * 64, :], po[0:64, :], pr_sb[:]
                    )

                norm_due = {}
                n_units = len(units)
                for i in range(n_units + LAG + NLAG + 4):
                    if i in norm_due:
                        for h in norm_due.pop(i):
                            emit_norm(h)
                    if i < n_units:
                        emit_scores(i)
                    ipv = i - LAG
                    if 0 <= ipv < n_units:
                        emit_pv(ipv)
                        p, st, lc = units[ipv]
                        if st == 7 and lc == 1:
                            norm_due.setdefault(i + NLAG, []).append(2 * p)
                            norm_due.setdefault(i + NLAG + 2, []).append(2 * p + 1)
                assert not norm_due and not pos and not ats

            # ================= phase D: output projection =================
            with tc.tile_pool(name="finps", bufs=4, space="PSUM") as finps:
                for ot in range(8):
                    for lc in range(2):
                        lcs = slice(lc * 512, (lc + 1) * 512)
                        pf = finps.tile([128, 512], dt.float32, name=f"pf{ot}_{lc}", tag="fin")
                        for p4 in range(4):
                            nc.tensor.matmul(
                                pf[:],
                                wp_t[:, p4 * 1024 + ot * 128 : p4 * 1024 + (ot + 1) * 128],
                                oT_t[p4][:, lcs],
                                start=(p4 == 0),
                                stop=(p4 == 3),
                            )
                        f_sb = ostage.tile([128, 512], dt.float32, name=f"fsb{ot}_{lc}", tag="fsb")
                        if use_bp:
                            nc.scalar.activation(
                                f_sb[:], pf[:], AF.Identity, bias=bp_t[:, ot : ot + 1]
                            )
                        else:
                            nc.vector.tensor_copy(f_sb[:], pf[:])
                        nc.sync.dma_start(
                            out_d[ot * 128 : (ot + 1) * 128, lcs], f_sb[:]
                        )

    _split_multiwait_instructions(nc)
    _NC_CACHE[key] = nc
    return nc


# ---------------------------------------------------------------- host side
def prep_inputs(inputs):
    """Shard + lay out the full inputs into 8 per-core input maps."""
    q = np.asarray(inputs["q"], np.float32)
    k = np.asarray(inputs["k"], np.float32)
    v = np.asarray(inputs["v"], np.float32)
    attn_mask = np.asarray(inputs["attn_mask"], bool)
    pos_bias = np.asarray(inputs["pos_bias"], np.float32)
    Wq = np.asarray(inputs["Wq"], np.float32)
    Wk = np.asarray(inputs["Wk"], np.float32)
    Wv = np.asarray(inputs["Wv"], np.float32)
    Wp = np.asarray(inputs["Wp"], np.float32)
    bq = np.asarray(inputs["bq"], np.float32)
    bk = np.asarray(inputs["bk"], np.float32)
    bv = np.asarray(inputs["bv"], np.float32)
    bp = np.asarray(inputs["bp"], np.float32)
    is_causal = int(np.asarray(inputs["is_causal"]))

    # effective mask: causal + row-any fix (matches the reference exactly)
    mask = attn_mask
    if is_causal:
        causal = np.tril(np.ones((L, L), bool))
        causal = np.pad(causal, ((0, 0), (S - L, 0)), constant_values=True)
        mask = mask & causal[None]
    row_any = mask.any(axis=-1, keepdims=True)
    mask = np.where(row_any, mask, True)  # (B, L, S)

    ident = np.eye(128, dtype=bf16)
    in_maps = []
    for core in range(8):
        b, hh = core // 2, core % 2
        c0 = hh * DIMC
        h0 = hh * NHC
        wq_c = (Wq[:, c0 : c0 + DIMC] * SCALE).astype(bf16)
        wk_c = Wk[:, c0 : c0 + DIMC].astype(bf16)
        wv_c = Wv[:, c0 : c0 + DIMC].astype(bf16)
        wp_c = Wp[c0 : c0 + DIMC, :].astype(bf16)
        pbT = (
            pos_bias[b, h0 : h0 + NHC]
            .transpose(0, 2, 1)
            .reshape(NHC * S, L)
            .astype(bf16)
        )
        in_maps.append(
            dict(
                qT=q[b].T.astype(bf16),
                kT=k[b].T.astype(bf16),
                vT=v[b].T.astype(bf16),
                wq=np.ascontiguousarray(wq_c),
                wk=np.ascontiguousarray(wk_c),
                wv=np.ascontiguousarray(wv_c),
                wp=np.ascontiguousarray(wp_c),
                pbT=np.ascontiguousarray(pbT),
                mfT=mask[b].T.astype(bf16),
                bq=np.ascontiguousarray((bq[c0 : c0 + DIMC] * SCALE)[None, :]),
                bk=np.ascontiguousarray(bk[c0 : c0 + DIMC][None, :]),
                bv=np.ascontiguousarray(bv[c0 : c0 + DIMC][None, :]),
                bp=(
                    np.ascontiguousarray(bp.reshape(8, 128).T)
                    if hh == 0
                    else np.zeros((128, 8), np.float32)
                ),
                ident=ident,
            )
        )
    return in_maps


def kernel(**inputs):
    global LAST_EXEC_NS
    from concourse.bass_utils import run_bass_kernel_spmd

    nc = build_nc(
        use_bq=bool(np.any(np.asarray(inputs["bq"]))),
        use_bk=bool(np.any(np.asarray(inputs["bk"]))),
        use_bv=bool(np.any(np.asarray(inputs["bv"]))),
        use_bp=bool(np.any(np.asarray(inputs["bp"]))),
    )
    in_maps = prep_inputs(inputs)
    kwargs = {}
    if TRACE and TRACE_DIR:
        kwargs["tmpdir"] = TRACE_DIR
    res = run_bass_kernel_spmd(
        nc, in_maps, core_ids=list(range(8)), trace=TRACE, **kwargs
    )
    LAST_EXEC_NS = res.exec_time_ns
    outs = res.results
    out = np.empty((B, L, DIM), np.float32)
    for b in range(B):
        out[b] = (outs[2 * b]["out"] + outs[2 * b + 1]["out"]).T
    return out


# revision 13
# speedup vs baseline: 1.5008x; 1.0521x over previous
"""Trainium2 Bass kernel for nn_AttentionBlock (B=4, L=S=1024, DIM=1024, NH=16).

Sharding: 8 cores = (batch b = core//2) x (head-half hh = core%2, 8 heads each).
Each core computes its batch's QKV projections restricted to its 512 feature
columns, attention for its 8 heads, and a partial output projection
(Wp row-slice); the host sums the two partials per batch.

Device layout is fully transposed ("T" = features/S on partitions) so no
on-device transposes are needed:
  qhT/khT (feat, L|S) from  Wslice.T @ xT ;  scoresT (S, L) = khT.T-slice @ qhT
  pos_bias enters PSUM via identity-matmul accumulation; mask is applied
  multiplicatively post-exp (host pre-merges causal+row-fix into the mask);
  softmax denominators ride a ones-column appended to V; normalization happens
  on the small (64, L) PV output via a rank-1 PE broadcast of 1/denom.
Compute dtype bf16 (f32 PSUM accumulation), f32 partial outputs.
"""
import contextlib
import ctypes
import sys
import types

import numpy as np
import ml_dtypes

bf16 = ml_dtypes.bfloat16

B, L, S, DIM, NH, DH = 4, 1024, 1024, 1024, 16, 64
NHC = 8           # heads per core
DIMC = 512        # feature columns per core
SCALE = 1.0 / np.sqrt(DH).astype(np.float32)

TRACE = False          # test.py flips this for profiling runs
TRACE_DIR = None
LAST_EXEC_NS = None


# ---------------------------------------------------------------- env setup
def _install_ntff_hook():
    if "antenv.axon_hooks" in sys.modules:
        return
    try:
        lib = ctypes.CDLL("/opt/axon/libaxon_pjrt.so")
        lib.axon_start_nrt_profile.argtypes = [
            ctypes.POINTER(ctypes.c_int64),
            ctypes.c_size_t,
        ]
        lib.axon_start_nrt_profile.restype = ctypes.c_int64
        lib.axon_stop_nrt_profile.argtypes = [ctypes.c_char_p]
        lib.axon_stop_nrt_profile.restype = ctypes.c_int64
    except OSError:
        return

    @contextlib.contextmanager
    def _hook(output_dir, device_ids):
        import jax

        jax.devices()
        if device_ids:
            ids = (ctypes.c_int64 * len(device_ids))(*device_ids)
            rc = lib.axon_start_nrt_profile(ids, len(device_ids))
        else:
            rc = lib.axon_start_nrt_profile(None, 0)
        if rc != 0:
            raise RuntimeError(f"axon_start_nrt_profile rc={rc}")
        try:
            yield
        finally:
            n = lib.axon_stop_nrt_profile(str(output_dir).encode())
            print(f"profile: {n} file(s) written to {output_dir}")

    mod = types.ModuleType("antenv.axon_hooks")
    mod.get_axon_ntff_profile_hook = lambda: _hook
    mod.set_axon_ntff_profile_hook = lambda h: None
    sys.modules["antenv.axon_hooks"] = mod


def _patch_tile_drain():
    from concourse import mybir
    from concourse.tile import TileContext, ScopedClock

    if getattr(TileContext, "_drain_split_patched", False):
        return

    def _drain_and_barrier(self, tick_clock, wait_clock):
        drain_inst = self.nc.sync.drain()
        wait_clock.add_sem_waits(
            drain_inst.ins, ScopedClock({None: tick_clock.global_clock})
        )
        waits = list(drain_inst.ins.sync_info.on_wait)
        if len(waits) > 1:
            drain_inst.ins.sync_info.on_wait = waits[:1]
            for w in waits[1:]:
                nop = self.nc.sync.nop()
                nop.ins.sync_info = mybir.SyncInfo(on_wait=[w], on_update=[])
        self.nc.all_engine_barrier()
        assert self.sems is not None
        popped = self.nc._tile_sem_poison_stack.pop()
        assert popped is self._sem_poison
        self.nc.clear_and_free_semaphores(list(self.sems.allocated().values()))
        self.nc.all_engine_barrier()

    TileContext._drain_and_barrier = _drain_and_barrier
    TileContext._drain_split_patched = True


def _split_multiwait_instructions(nc):
    """This container's walrus rejects >1 sync wait per instruction; hoist
    extras onto same-engine NOPs placed right before the instruction."""
    from concourse import mybir

    n_split = 0
    for fn in nc.m.functions:
        for bb in fn.blocks:
            out = []
            for inst in bb.instructions:
                si = inst.sync_info
                waits = list(si.on_wait) if si is not None else []
                if len(waits) > 1:
                    for w in waits[:-1]:
                        n_split += 1
                        out.append(
                            mybir.InstNoOp(
                                name=f"waitsplit-{n_split}-{inst.name}",
                                engine=inst.engine,
                                bass_nofuse=True,
                                sync_info=mybir.SyncInfo(on_wait=[w], on_update=[]),
                            )
                        )
                    si.on_wait = waits[-1:]
                out.append(inst)
            if n_split:
                bb.instructions = out


# ---------------------------------------------------------------- builder
_NC_CACHE = {}


def build_nc(use_bq=False, use_bk=False, use_bv=False, use_bp=False):
    key = (use_bq, use_bk, use_bv, use_bp)
    if key in _NC_CACHE:
        return _NC_CACHE[key]
    _install_ntff_hook()
    _patch_tile_drain()
    import concourse.bass as bass
    import concourse.tile as tile
    from concourse import mybir

    dt = mybir.dt
    AF = mybir.ActivationFunctionType

    nc = bass.Bass("TRN2", target_bir_lowering=False, debug=False, num_devices=8)

    qT_d = nc.declare_dram_parameter("qT", (DIM, L), dt.bfloat16, isOutput=False)
    kT_d = nc.declare_dram_parameter("kT", (DIM, S), dt.bfloat16, isOutput=False)
    vT_d = nc.declare_dram_parameter("vT", (DIM, S), dt.bfloat16, isOutput=False)
    wq_d = nc.declare_dram_parameter("wq", (DIM, DIMC), dt.bfloat16, isOutput=False)
    wk_d = nc.declare_dram_parameter("wk", (DIM, DIMC), dt.bfloat16, isOutput=False)
    wv_d = nc.declare_dram_parameter("wv", (DIM, DIMC), dt.bfloat16, isOutput=False)
    wp_d = nc.declare_dram_parameter("wp", (DIMC, DIM), dt.bfloat16, isOutput=False)
    pb_d = nc.declare_dram_parameter("pbT", (NHC * S, L), dt.bfloat16, isOutput=False)
    mf_d = nc.declare_dram_parameter("mfT", (S, L), dt.bfloat16, isOutput=False)
    bq_d = nc.declare_dram_parameter("bq", (1, DIMC), dt.float32, isOutput=False)
    bk_d = nc.declare_dram_parameter("bk", (1, DIMC), dt.float32, isOutput=False)
    bv_d = nc.declare_dram_parameter("bv", (1, DIMC), dt.float32, isOutput=False)
    bp_d = nc.declare_dram_parameter("bp", (128, 8), dt.float32, isOutput=False)
    id_d = nc.declare_dram_parameter("ident", (128, 128), dt.bfloat16, isOutput=False)
    out_d = nc.declare_dram_parameter("out", (DIM, L), dt.float32, isOutput=True)

    with tile.TileContext(nc) as tc:
        with (
            tc.tile_pool(name="consts", bufs=1) as consts,
            tc.tile_pool(name="w", bufs=1) as wpool,
            tc.tile_pool(name="mf", bufs=1) as mfpool,
            tc.tile_pool(name="heads", bufs=1) as heads,
            tc.tile_pool(name="stage", bufs=1) as stage,
            tc.tile_pool(name="ostage", bufs=4) as ostage,
            tc.tile_pool(name="pb", bufs=4) as pbp,
            tc.tile_pool(name="attn", bufs=12) as attnp,
        ):
            ones_t = consts.tile([128, 64], dt.bfloat16)
            nc.gpsimd.memset(ones_t[:], 1.0)
            if use_bq:
                bq_t = consts.tile([1, DIMC], dt.float32)
                nc.sync.dma_start(bq_t[:], bq_d[:])
            if use_bk:
                bk_t = consts.tile([1, DIMC], dt.float32)
                nc.sync.dma_start(bk_t[:], bk_d[:])
            if use_bv:
                bv_t = consts.tile([1, DIMC], dt.float32)
                nc.sync.dma_start(bv_t[:], bv_d[:])
                ones_f = consts.tile([1, 128], dt.float32)
                nc.gpsimd.memset(ones_f[:], 1.0)
            if use_bq or use_bk:
                ones_r = consts.tile([1, 512], dt.float32)
                nc.gpsimd.memset(ones_r[:], 1.0)

            # big consolidated weight tiles: w*[:, dt*512 + p*128 + ...] = W[dt*128+p-row, col]
            wq_t = wpool.tile([128, 8 * DIMC], dt.bfloat16, name="wqb", tag="wqb")
            wk_t = wpool.tile([128, 8 * DIMC], dt.bfloat16, name="wkb", tag="wkb")
            wv_t = wpool.tile([128, 8 * DIMC], dt.bfloat16, name="wvb", tag="wvb")
            wp_t = wpool.tile([128, 4 * DIM], dt.bfloat16, name="wpb", tag="wpb")
            mf_t = mfpool.tile([128, 8 * L], dt.bfloat16, name="mfb", tag="mfb")

            qh_t = [heads.tile([128, L], dt.bfloat16, name=f"qh{i}", tag=f"qh{i}") for i in range(4)]
            kh_t = [heads.tile([128, S], dt.bfloat16, name=f"kh{i}", tag=f"kh{i}") for i in range(4)]
            vh_t = [heads.tile([128, NHC * 65], dt.bfloat16, name=f"vh{i}", tag=f"vh{i}") for i in range(8)]
            oT_t = [heads.tile([128, L], dt.bfloat16, name=f"oT{i}", tag=f"oT{i}") for i in range(4)]

            def load_big(tile_ap, dram, rows, cols, chunks=1):
                # tile[:, a*cols + c] = dram[a*128 + p, c]
                n_a = rows // 128
                a_per = n_a // chunks
                for ch in range(chunks):
                    nc.sync.dma_start(
                        tile_ap[:, ch * a_per * cols : (ch + 1) * a_per * cols]
                        .rearrange("p (a c) -> p a c", c=cols),
                        dram[ch * a_per * 128 : (ch + 1) * a_per * 128, :]
                        .rearrange("(a p) c -> p a c", p=128),
                    )

            # ================= phase B: projections =================
            with (
                tc.tile_pool(name="xT", bufs=1) as xTp,
                tc.tile_pool(name="projps", bufs=4, space="PSUM") as pps,
            ):
                load_big(wq_t, wq_d, DIM, DIMC)
                xq = []
                for dtile in range(8):
                    x_t = xTp.tile([128, 1024], dt.bfloat16, name=f"xq{dtile}", tag=f"xq{dtile}")
                    nc.sync.dma_start(x_t[:], qT_d[dtile * 128 : (dtile + 1) * 128, :])
                    xq.append(x_t)
                load_big(wk_t, wk_d, DIM, DIMC)
                xk = []
                for dtile in range(8):
                    x_t = xTp.tile([128, 1024], dt.bfloat16, name=f"xk{dtile}", tag=f"xk{dtile}")
                    nc.sync.dma_start(x_t[:], kT_d[dtile * 128 : (dtile + 1) * 128, :])
                    xk.append(x_t)
                xb = {"q": xq, "k": xk}
                id_t = consts.tile([128, 128], dt.bfloat16)
                nc.sync.dma_start(id_t[:], id_d[:])
                load_big(mf_t, mf_d, S, L)

                # Q+K projections interleaved per head-pair so pair 0 finishes
                # early and attention can start while pairs 1-3 project
                for p in range(4):
                    psq = pps.tile([128, 1024], dt.float32, name=f"psq{p}", tag="ps")
                    psk = pps.tile([128, 1024], dt.float32, name=f"psk{p}", tag="ps")
                    for dtile in range(8):
                        for ps, w_t, x_l in ((psq, wq_t, xb["q"]), (psk, wk_t, xb["k"])):
                            for lc in range(2):
                                nc.tensor.matmul(
                                    ps[:, lc * 512 : (lc + 1) * 512],
                                    w_t[:, dtile * 512 + p * 128 : dtile * 512 + (p + 1) * 128],
                                    x_l[dtile][:, lc * 512 : (lc + 1) * 512],
                                    start=(dtile == 0),
                                    stop=(dtile == 7)
                                    and not (use_bq if w_t is wq_t else use_bk),
                                )
                    if use_bq:
                        for lc in range(2):
                            nc.tensor.matmul(
                                psq[:, lc * 512 : (lc + 1) * 512],
                                bq_t[0:1, p * 128 : (p + 1) * 128],
                                ones_r[0:1, 0:512],
                                start=False, stop=True,
                            )
                    if use_bk:
                        for lc in range(2):
                            nc.tensor.matmul(
                                psk[:, lc * 512 : (lc + 1) * 512],
                                bk_t[0:1, p * 128 : (p + 1) * 128],
                                ones_r[0:1, 0:512],
                                start=False, stop=True,
                            )
                    nc.vector.tensor_copy(qh_t[p][:], psq[:])
                    nc.vector.tensor_copy(kh_t[p][:], psk[:])

                # V: normal layout (S on partitions), ones column interleaved
                load_big(wv_t, wv_d, DIM, DIMC)
                xv = []
                for dtile in range(8):
                    x_t = xTp.tile([128, 1024], dt.bfloat16, name=f"xv{dtile}", tag=f"xq{dtile}")
                    nc.sync.dma_start(x_t[:], vT_d[dtile * 128 : (dtile + 1) * 128, :])
                    xv.append(x_t)
                psv = [pps.tile([128, 1024], dt.float32, name=f"psv{i}", tag="ps") for i in range(4)]
                for dtile in range(8):
                    for i in range(4):
                        for half in range(2):
                            st = 2 * i + half
                            nc.tensor.matmul(
                                psv[i][:, half * 512 : (half + 1) * 512],
                                xv[dtile][:, st * 128 : (st + 1) * 128],
                                wv_t[:, dtile * 512 : (dtile + 1) * 512],
                                start=(dtile == 0),
                                stop=(dtile == 7) and not use_bv,
                            )
                for i in range(4):
                    for half in range(2):
                        st = 2 * i + half
                        if use_bv:
                            nc.tensor.matmul(
                                psv[i][:, half * 512 : (half + 1) * 512],
                                ones_f[0:1, 0:128],
                                bv_t[:],
                                start=False, stop=True,
                            )
                        nc.gpsimd.memset(vh_t[st][:], 1.0)
                        nc.vector.tensor_copy(
                            vh_t[st]
                            .rearrange("p (h x) -> p h x", x=65)[:, :, 0:64],
                            psv[i][:, half * 512 : (half + 1) * 512].rearrange(
                                "p (h x) -> p h x", x=64
                            ),
                        )

            # late loads (phase D)
            load_big(wp_t, wp_d, DIMC, DIM)
            bp_t = consts.tile([128, 8], dt.float32)
            if use_bp:
                nc.sync.dma_start(bp_t[:], bp_d[:])

            # ====== phase C: attention; pair-adjacent scores, lc-granular pipeline ======
            with (
                tc.tile_pool(name="scps", bufs=4, space="PSUM") as scps,
                tc.tile_pool(name="pvps", bufs=2, space="PSUM") as pvps,
            ):
                LAG = 3
                NLAG = 2
                units = [
                    (p, st, lc) for p in range(4) for st in range(8) for lc in range(2)
                ]
                ats = {}
                pos = {}
                pbs = {}

                def emit_scores(i):
                    p, st, lc = units[i]
                    lcs = slice(lc * 512, (lc + 1) * 512)
                    if st == 0 and lc == 0:
                        for j in range(2):
                            h = 2 * p + j
                            pb_t = pbp.tile([128, 8 * L], dt.bfloat16, name=f"pb{h}", tag="pb")
                            load_big(pb_t, pb_d[h * S : (h + 1) * S, :], S, L)
                            pbs[h] = pb_t
                            pos[h] = pvps.tile([65, L], dt.float32, name=f"po{h}", tag="pv")
                    pss = []
                    for j in range(2):
                        h = 2 * p + j
                        ps = scps.tile([128, 512], dt.float32, name=f"sc{h}_{st}_{lc}", tag="sc")
                        nc.tensor.matmul(
                            ps[:], id_t[:],
                            pbs[h][:, st * 1024 + lc * 512 : st * 1024 + (lc + 1) * 512],
                            start=True, stop=False,
                        )
                        pss.append(ps)
                    for j in range(2):
                        h = 2 * p + j
                        jj = j * 64
                        nc.tensor.matmul(
                            pss[j][:],
                            kh_t[p][jj : jj + 64, st * 128 : (st + 1) * 128],
                            qh_t[p][jj : jj + 64, lcs],
                            start=False, stop=True,
                            tile_position=(jj, 0),
                        )
                    for j in range(2):
                        h = 2 * p + j
                        at = attnp.tile([128, 512], dt.bfloat16, name=f"at{h}_{st}_{lc}", tag="attn")
                        nc.scalar.activation(at[:], pss[j][:], AF.Exp)
                        nc.vector.tensor_mul(
                            at[:], at[:], mf_t[:, st * 1024 + lc * 512 : st * 1024 + (lc + 1) * 512]
                        )
                        ats[(h, st, lc)] = at

                def emit_pv(i):
                    p, st, lc = units[i]
                    lcs = slice(lc * 512, (lc + 1) * 512)
                    for j in range(2):
                        h = 2 * p + j
                        at = ats.pop((h, st, lc))
                        nc.tensor.matmul(
                            pos[h][:, lcs],
                            vh_t[st][:, h * 65 : h * 65 + 65],
                            at[:],
                            start=(st == 0),
                            stop=(st == 7),
                        )

                def emit_norm(h):
                    p, j = h // 2, h % 2
                    po = pos.pop(h)
                    pbs.pop(h, None)
                    lnr = stage.tile([65, L], dt.float32, name=f"lnr{h}", tag="lnr")
                    nc.scalar.activation(lnr[64:65, :], po[64:65, :], AF.Ln)
                    rec_bf = stage.tile([65, L], dt.bfloat16, name=f"recbf{h}", tag="recbf")
                    nc.scalar.activation(
                        rec_bf[64:65, :], lnr[64:65, :], AF.Exp, scale=-1.0
                    )
                    pr = scps.tile([128, 512], dt.float32, name=f"pr{h}a", tag="sc")
                    pr2 = scps.tile([128, 512], dt.float32, name=f"pr{h}b", tag="sc")
                    for lc, prt in ((0, pr), (1, pr2)):
                        nc.tensor.matmul(
                            prt[0:64, :],
                            ones_t[64:65, 0:64],
                            rec_bf[64:65, lc * 512 : (lc + 1) * 512],
                            start=True, stop=True,
                        )
                    pr_sb = stage.tile([64, L], dt.float32, name=f"prsb{h}", tag="prsb")
                    nc.vector.tensor_copy(pr_sb[:, 0:512], pr[0:64, :])
                    nc.vector.tensor_copy(pr_sb[:, 512:1024], pr2[0:64, :])
                    nc.vector.tensor_mul(
                        oT_t[p][j * 64 : (j + 1) * 64, :], po[0:64, :], pr_sb[:]
                    )

                norm_due = {}
                n_units = len(units)
                for i in range(n_units + LAG + NLAG + 4):
                    if i in norm_due:
                        for h in norm_due.pop(i):
                            emit_norm(h)
                    if i < n_units:
                        emit_scores(i)
                    ipv = i - LAG
                    if 0 <= ipv < n_units:
                        emit_pv(ipv)
                        p, st, lc = units[ipv]
                        if st == 7 and lc == 1:
                            norm_due.setdefault(i + NLAG, []).append(2 * p)
                            norm_due.setdefault(i + NLAG + 2, []).append(2 * p + 1)
                assert not norm_due and not pos and not ats

            # ================= phase D: output projection =================
            with tc.tile_pool(name="finps", bufs=4, space="PSUM") as finps:
                for ot in range(8):
                    for lc in range(2):
                        lcs = slice(lc * 512, (lc + 1) * 512)
                        pf = finps.tile([128, 512], dt.float32, name=f"pf{ot}_{lc}", tag="fin")
                        for p4 in range(4):
                            nc.tensor.matmul(
                                pf[:],
                                wp_t[:, p4 * 1024 + ot * 128 : p4 * 1024 + (ot + 1) * 128],
                                oT_t[p4][:, lcs],
                                start=(p4 == 0),
                                stop=(p4 == 3),
                            )
                        f_sb = ostage.tile([128, 512], dt.float32, name=f"fsb{ot}_{lc}", tag="fsb")
                        if use_bp:
                            nc.scalar.activation(
                                f_sb[:], pf[:], AF.Identity, bias=bp_t[:, ot : ot + 1]
                            )
                        else:
                            nc.vector.tensor_copy(f_sb[:], pf[:])
                        nc.sync.dma_start(
                            out_d[ot * 128 : (ot + 1) * 128, lcs], f_sb[:]
                        )

    _split_multiwait_instructions(nc)
    _NC_CACHE[key] = nc
    return nc


# ---------------------------------------------------------------- host side
def prep_inputs(inputs):
    """Shard + lay out the full inputs into 8 per-core input maps."""
    q = np.asarray(inputs["q"], np.float32)
    k = np.asarray(inputs["k"], np.float32)
    v = np.asarray(inputs["v"], np.float32)
    attn_mask = np.asarray(inputs["attn_mask"], bool)
    pos_bias = np.asarray(inputs["pos_bias"], np.float32)
    Wq = np.asarray(inputs["Wq"], np.float32)
    Wk = np.asarray(inputs["Wk"], np.float32)
    Wv = np.asarray(inputs["Wv"], np.float32)
    Wp = np.asarray(inputs["Wp"], np.float32)
    bq = np.asarray(inputs["bq"], np.float32)
    bk = np.asarray(inputs["bk"], np.float32)
    bv = np.asarray(inputs["bv"], np.float32)
    bp = np.asarray(inputs["bp"], np.float32)
    is_causal = int(np.asarray(inputs["is_causal"]))

    # effective mask: causal + row-any fix (matches the reference exactly)
    mask = attn_mask
    if is_causal:
        causal = np.tril(np.ones((L, L), bool))
        causal = np.pad(causal, ((0, 0), (S - L, 0)), constant_values=True)
        mask = mask & causal[None]
    row_any = mask.any(axis=-1, keepdims=True)
    mask = np.where(row_any, mask, True)  # (B, L, S)

    ident = np.eye(128, dtype=bf16)
    in_maps = []
    for core in range(8):
        b, hh = core // 2, core % 2
        c0 = hh * DIMC
        h0 = hh * NHC
        wq_c = (Wq[:, c0 : c0 + DIMC] * SCALE).astype(bf16)
        wk_c = Wk[:, c0 : c0 + DIMC].astype(bf16)
        wv_c = Wv[:, c0 : c0 + DIMC].astype(bf16)
        wp_c = Wp[c0 : c0 + DIMC, :].astype(bf16)
        pbT = (
            pos_bias[b, h0 : h0 + NHC]
            .transpose(0, 2, 1)
            .reshape(NHC * S, L)
            .astype(bf16)
        )
        in_maps.append(
            dict(
                qT=q[b].T.astype(bf16),
                kT=k[b].T.astype(bf16),
                vT=v[b].T.astype(bf16),
                wq=np.ascontiguousarray(wq_c),
                wk=np.ascontiguousarray(wk_c),
                wv=np.ascontiguousarray(wv_c),
                wp=np.ascontiguousarray(wp_c),
                pbT=np.ascontiguousarray(pbT),
                mfT=mask[b].T.astype(bf16),
                bq=np.ascontiguousarray((bq[c0 : c0 + DIMC] * SCALE)[None, :]),
                bk=np.ascontiguousarray(bk[c0 : c0 + DIMC][None, :]),
                bv=np.ascontiguousarray(bv[c0 : c0 + DIMC][None, :]),
                bp=(
                    np.ascontiguousarray(bp.reshape(8, 128).T)
                    if hh == 0
                    else np.zeros((128, 8), np.float32)
                ),
                ident=ident,
            )
        )
    return in_maps


def kernel(**inputs):
    global LAST_EXEC_NS
    from concourse.bass_utils import run_bass_kernel_spmd

    nc = build_nc(
        use_bq=bool(np.any(np.asarray(inputs["bq"]))),
        use_bk=bool(np.any(np.asarray(inputs["bk"]))),
        use_bv=bool(np.any(np.asarray(inputs["bv"]))),
        use_bp=bool(np.any(np.asarray(inputs["bp"]))),
    )
    in_maps = prep_inputs(inputs)
    kwargs = {}
    if TRACE and TRACE_DIR:
        kwargs["tmpdir"] = TRACE_DIR
    res = run_bass_kernel_spmd(
        nc, in_maps, core_ids=list(range(8)), trace=TRACE, **kwargs
    )
    LAST_EXEC_NS = res.exec_time_ns
    outs = res.results
    out = np.empty((B, L, DIM), np.float32)
    for b in range(B):
        out[b] = (outs[2 * b]["out"] + outs[2 * b + 1]["out"]).T
    return out
